# revision 48
# baseline (speedup 1.0000x reference)
"""Trainium2 Bass kernel for nn_Encoder_70781061038947.

Math: row b's output depends on x[b, :] only through its 16 sign bits.
P_b(t_m) = prod_k (t_m - z_k^{(b_k)}) over the 17th roots of unity t_m.
Log-linearize: log P_b(t_m) = C(m) + sum_k b_k D(k, m) with b_k in {0,1},
so one bf16 hi/lo matmul against a 128x272 block-diagonal table computes
all 8 tiles' complex logs at once.  Then E = exp(re) * cis(2*pi*im-turns)
via ACT Exp/Sin (phases range-reduced with the float round trick; the cos
window uses cos(2*pi*|d|) = sin(pi/2 - 2*pi*|d|) to stay inside Sin's
[-pi, pi] domain).  Parseval: S = sum_m exp(re)^2, fac = rsqrt(S) via
bit-hack seed + 2 Newton rounds on DVE/Pool (no ACT table switch).  The x17
normalization constant is folded into the inverse-DFT matrix, which runs
as bf16 hi+lo matmuls on transposed evals.

ACT ordering is function-major within each of two super-phases (Sin-set
ops, then Exp-set ops, gated via zero-valued bias tiles) so the activation
table loads only four times and back(A) overlaps front(B).

Sharding: pure data parallel over B across 8 cores (32768 rows each); the
small tables derived from shuffle_vector (host FLOPs independent of B) are
replicated inputs.
"""

import numpy as np
import ml_dtypes

import concourse.bacc as bacc
import concourse.bass as bass
import concourse.mybir as mybir
import concourse.bass_utils as bass_utils
import concourse.tile as tile

B = 262144
K = 16
M = 17                      # evaluation points (17th roots of unity)
W = 2 * M                   # 34 f32 per output row
NCORES = 8
RPC = B // NCORES           # 32768 rows per core
P = 128
CPB = RPC // P              # 256 rows per partition
TPC = 8                     # tiles (row-columns) per chunk
NCHUNK = CPB // TPC         # 32 chunks
CPG = 4                     # chunks per group
NGROUP = NCHUNK // CPG      # 8 groups
FT = TPC * K                # 128 free cols of one x chunk
FO = TPC * W                # 272 free cols of one out chunk
GM = CPG * TPC * M          # 544: packed (c,t,m) free width per group

MAGIC = float(0x5F3759DF)
RND = 12582912.0            # 1.5 * 2^23: float round trick

_cached = None


def _tables(shuffle_vector: np.ndarray):
    sv = np.asarray(shuffle_vector, dtype=np.float64)
    R = np.sqrt(1.0 + np.sin(np.pi / K))
    t = np.exp(2j * np.pi * np.arange(M) / M)
    bf16 = ml_dtypes.bfloat16

    # complex logs of the two per-bit root choices at each eval point
    z1 = R * np.exp(1j * sv)              # bit=1 roots (K,)
    z0 = (1.0 / R) * np.exp(1j * sv)      # bit=0 roots
    L1 = np.log(t[None, :] - z1[:, None])   # (K, M) complex
    L0 = np.log(t[None, :] - z0[:, None])
    D = L1 - L0                             # per-bit delta
    C = L0.sum(axis=0)                      # constant part (M,)

    def turns(x):
        v = x / (2 * np.pi)
        return v - np.round(v)              # [-0.5, 0.5]

    # block-diagonal [128, 272]: row t*16+k, col t*34+e (e<17 re, e>=17 im-turns)
    D8 = np.zeros((P, FO), np.float64)
    for tt in range(TPC):
        for k in range(K):
            D8[tt * K + k, tt * W: tt * W + M] = D[k].real
            D8[tt * K + k, tt * W + M: tt * W + W] = turns(D[k].imag)
    Crow = np.zeros((1, FO), np.float64)
    for tt in range(TPC):
        Crow[0, tt * W: tt * W + M] = C.real
        Crow[0, tt * W + M: tt * W + W] = turns(C.imag)

    D8hi = D8.astype(bf16)
    D8lo = (D8 - D8hi.astype(np.float64)).astype(bf16)
    Chi = Crow.astype(bf16)
    Clo = (Crow - Chi.astype(np.float64)).astype(bf16)

    # inverse-DFT with the x17 normalization folded in (|w| = 1)
    w2r = np.zeros((W, W), np.float64)
    for m in range(M):
        for d in range(M):
            w = np.exp(-2j * np.pi * ((K - d) * m) / M)
            w2r[m, 2 * d] = w.real
            w2r[m, 2 * d + 1] = w.imag
            w2r[M + m, 2 * d] = -w.imag
            w2r[M + m, 2 * d + 1] = w.real
    w2r3 = np.zeros((3 * W, 3 * W), np.float64)
    for j in range(3):
        w2r3[j * W:(j + 1) * W, j * W:(j + 1) * W] = w2r
    w2r2 = np.zeros((2 * W, 2 * W), np.float64)
    for j in range(2):
        w2r2[j * W:(j + 1) * W, j * W:(j + 1) * W] = w2r

    w3hi = w2r3.astype(bf16)
    w2hi = w2r2.astype(bf16)

    ident_bf = np.eye(P, dtype=bf16)
    ones_row = np.ones((1, P), bf16)

    return {
        "d8hi": D8hi, "d8lo": D8lo, "chi": Chi, "clo": Clo,
        "w3hi": w3hi, "w2hi": w2hi, "identb": ident_bf, "onesr": ones_row,
    }


def _build_module(rpc=RPC):
    cpb = rpc // P
    nchunk = cpb // TPC
    ngroup = nchunk // CPG
    f32 = mybir.dt.float32
    bf = mybir.dt.bfloat16
    u32 = mybir.dt.uint32
    AF = mybir.ActivationFunctionType
    OP = mybir.AluOpType
    TWOPI = float(2 * np.pi)

    nc = bacc.Bacc("TRN2", target_bir_lowering=False, debug=False)
    # x pre-transposed on host: xt[(t,k), ci*128 + p] = x[p*cpb + ci*TPC + t, k]
    xt_d = nc.dram_tensor("xt", [P, nchunk * P], bf, kind="ExternalInput")
    d8hi_d = nc.dram_tensor("d8hi", [P, FO], bf, kind="ExternalInput")
    d8lo_d = nc.dram_tensor("d8lo", [P, FO], bf, kind="ExternalInput")
    chi_d = nc.dram_tensor("chi", [1, FO], bf, kind="ExternalInput")
    clo_d = nc.dram_tensor("clo", [1, FO], bf, kind="ExternalInput")
    onesr_d = nc.dram_tensor("onesr", [1, P], bf, kind="ExternalInput")
    w3hi_d = nc.dram_tensor("w3hi", [3 * W, 3 * W], bf, kind="ExternalInput")
    w2hi_d = nc.dram_tensor("w2hi", [2 * W, 2 * W], bf, kind="ExternalInput")
    identb_d = nc.dram_tensor("identb", [P, P], bf, kind="ExternalInput")
    out_d = nc.dram_tensor("out", [rpc, W], bf, kind="ExternalOutput")

    # row (p*cpb + c) -> partition p, column c
    out_v = out_d.ap().rearrange("(p c) e -> p (c e)", p=P)  # [128, cpb*34]

    with tile.TileContext(nc) as tc:
        with (
            tc.tile_pool(name="const", bufs=1) as cp,
            tc.tile_pool(name="sb", bufs=2) as sp,
            tc.tile_pool(name="keep", bufs=1) as kp,
            tc.tile_pool(name="ps", bufs=1, space="PSUM") as pp,
        ):
            half_x = nchunk * P // 2
            d8hi = cp.tile([P, FO], bf)
            nc.sync.dma_start(out=d8hi[:], in_=d8hi_d.ap())
            d8lo = cp.tile([P, FO], bf)
            nc.sync.dma_start(out=d8lo[:], in_=d8lo_d.ap())
            xt = cp.tile([P, nchunk * P], bf)
            g0w = CPG * P
            nc.sync.dma_start(out=xt[:, 0:g0w], in_=xt_d.ap()[:, 0:g0w])
            chi = cp.tile([1, FO], bf)
            nc.sync.dma_start(out=chi[:], in_=chi_d.ap())
            clo = cp.tile([1, FO], bf)
            nc.sync.dma_start(out=clo[:], in_=clo_d.ap())
            onesr = cp.tile([1, P], bf)
            nc.sync.dma_start(out=onesr[:], in_=onesr_d.ap())
            identb = cp.tile([P, P], bf)
            nc.sync.dma_start(out=identb[:], in_=identb_d.ap())
            w3hi = cp.tile([3 * W, 3 * W], bf)
            nc.sync.dma_start(out=w3hi[:], in_=w3hi_d.ap())
            w2hi = cp.tile([2 * W, 2 * W], bf)
            nc.sync.dma_start(out=w2hi[:], in_=w2hi_d.ap())
            nc.sync.dma_start(out=xt[:, g0w:half_x], in_=xt_d.ap()[:, g0w:half_x])
            nc.sync.dma_start(out=xt[:, half_x:], in_=xt_d.ap()[:, half_x:])
            halfpi = cp.tile([P, 1], f32)
            nc.gpsimd.memset(halfpi[:], float(np.pi / 2))
            # dummy Sin: pull the first activation table load into the ramp
            dummy = cp.tile([P, 1], f32)
            nc.scalar.activation(out=dummy[:], in_=halfpi[:], func=AF.Sin)

            ls = [None] * ngroup      # staged logsum [128, CPG*FO] f32
            sinv = [None] * ngroup    # sin [128, GM] f32
            cosv = [None] * ngroup    # cos [128, GM] f32

            # ---------------- front: signs, log matmuls, trig -------------
            def front(g, sin_bias, cos_bias, dma_eng):
                s01 = sp.tile([P, CPG * P], bf, tag="s01")
                nc.gpsimd.tensor_scalar(
                    out=s01[:], in0=xt[:, g * CPG * P:(g + 1) * CPG * P],
                    scalar1=0.0, scalar2=None, op0=OP.is_gt)

                ls_g = kp.tile([P, CPG * FO], f32, tag=f"ls{g}", name=f"ls{g}")
                for cp2 in range(CPG // 2):
                    lsp = pp.tile([P, 1024], f32, tag="lsp", bufs=2)
                    for c2 in range(2):
                        c = 2 * cp2 + c2
                        lv = lsp[:, c2 * 512:c2 * 512 + FO]
                        nc.tensor.matmul(out=lv,
                                         lhsT=s01[:, c * P:(c + 1) * P],
                                         rhs=d8hi[:], start=True, stop=False)
                        nc.tensor.matmul(out=lv,
                                         lhsT=s01[:, c * P:(c + 1) * P],
                                         rhs=d8lo[:], start=False, stop=False)
                        nc.tensor.matmul(out=lv, lhsT=onesr[:], rhs=chi[:],
                                         start=False, stop=False)
                        nc.tensor.matmul(out=lv, lhsT=onesr[:], rhs=clo[:],
                                         start=False, stop=True)
                    # stage both chunks to SBUF, alternating DVE / ACT
                    s_out = (ls_g[:, 2 * cp2 * FO:(2 * cp2 + 2) * FO]
                             .rearrange("p (c h) -> p c h", c=2))
                    s_in = lsp[:].rearrange("p (c h) -> p c h", c=2)[:, :, 0:FO]
                    if (g + cp2) % 2 == 0:
                        nc.vector.tensor_copy(out=s_out, in_=s_in)
                    else:
                        nc.scalar.activation(out=s_out, in_=s_in, func=AF.Copy)
                ls[g] = ls_g

                # phases: u = im-turns part, strided [128, (c,t), 17]
                lsv = ls_g[:].rearrange("p (n e) -> p n e", e=W)
                u = lsv[:, :, M:W]
                kf = sp.tile([P, GM], f32, tag="kf")
                kfv = kf[:].rearrange("p (n e) -> p n e", e=M)
                nc.vector.tensor_scalar(
                    out=kfv, in0=u, scalar1=RND, scalar2=RND,
                    op0=OP.add, op1=OP.subtract)
                d_g = sp.tile([P, GM], f32, tag="d")
                dv = d_g[:].rearrange("p (n e) -> p n e", e=M)
                nc.gpsimd.tensor_tensor(out=dv, in0=u, in1=kfv, op=OP.subtract)
                dabs = sp.tile([P, GM], f32, tag="dabs")
                nc.vector.tensor_scalar(
                    out=dabs[:].bitcast(u32), in0=d_g[:].bitcast(u32),
                    scalar1=int(0x7fffffff), scalar2=None, op0=OP.bitwise_and)

                sv_g = kp.tile([P, GM], bf, tag=f"sin{g}", name=f"sin{g}")
                nc.scalar.activation(out=sv_g[:], in_=d_g[:], func=AF.Sin,
                                     scale=TWOPI, bias=sin_bias)
                cv_g = kp.tile([P, GM], bf, tag=f"cos{g}", name=f"cos{g}")
                nc.scalar.activation(out=cv_g[:], in_=dabs[:], func=AF.Sin,
                                     scale=-TWOPI, bias=cos_bias)
                sinv[g] = sv_g
                cosv[g] = cv_g

            # ---------------- back: exp, normalize, iDFT, out --------------
            def back_exp(g, zgate):
                """Exp + Square + S-reduce for group g; returns expv tile."""
                lsv = ls[g][:].rearrange("p (n e) -> p n e", e=W)
                expv = sp.tile([P, GM], bf, tag=f"expv{g % 2}")
                ev = expv[:].rearrange("p (n e) -> p n e", e=M)
                nc.scalar.activation(out=ev, in_=lsv[:, :, 0:M], func=AF.Exp,
                                     bias=zgate[:])
                sq = sp.tile([P, GM], bf, tag="sq")
                nc.gpsimd.tensor_tensor(out=sq[:], in0=expv[:], in1=expv[:],
                                        op=OP.mult)
                return expv, sq

            def back_tail(g, expv, fac, fbase, late=False):
                ev = expv[:].rearrange("p (n e) -> p n e", e=M)
                vc = sp.tile([P, CPG * FO], bf, tag="vc")
                vcv = vc[:].rearrange("p (n e) -> p n e", e=W)
                nc.gpsimd.tensor_tensor(
                    out=vcv[:, :, 0:M],
                    in0=cosv[g][:].rearrange("p (n e) -> p n e", e=M),
                    in1=ev, op=OP.mult)
                nc.gpsimd.tensor_tensor(
                    out=vcv[:, :, M:W],
                    in0=sinv[g][:].rearrange("p (n e) -> p n e", e=M),
                    in1=ev, op=OP.mult)

                out_sb = sp.tile([P, CPG * FO], bf, tag="osb")
                widths = [3 * W, 3 * W, 2 * W]
                for cp2 in range(CPG // 2):
                    # two chunks share one PSUM bank for vcT and o_ps
                    vcT = pp.tile([3 * W, 6 * P], bf, tag="vcT", bufs=2)
                    for c2 in range(2):
                        vcc = vc[:, (2 * cp2 + c2) * FO:(2 * cp2 + c2 + 1) * FO]
                        for j, wdt in enumerate(widths):
                            nc.tensor.transpose(
                                out=vcT[0:wdt, (2 * j + c2) * P:(2 * j + c2 + 1) * P],
                                in_=vcc[:, j * 3 * W: j * 3 * W + wdt],
                                identity=identb[:])
                    vcT_sb = sp.tile([3 * W, 6 * P], bf, tag="vcTs")
                    if late:
                        nc.scalar.activation(out=vcT_sb[0:3 * W, 0:4 * P],
                                             in_=vcT[0:3 * W, 0:4 * P],
                                             func=AF.Copy)
                    else:
                        nc.vector.tensor_copy(out=vcT_sb[0:3 * W, 0:4 * P],
                                              in_=vcT[0:3 * W, 0:4 * P])
                    nc.scalar.activation(out=vcT_sb[0:2 * W, 4 * P:6 * P],
                                         in_=vcT[0:2 * W, 4 * P:6 * P],
                                         func=AF.Copy)

                    o_ps = pp.tile([P, 1024], f32, tag="o", bufs=1)
                    for c2 in range(2):
                        ob = c2 * 512
                        nc.tensor.matmul(
                            out=o_ps[:, ob:ob + 3 * W],
                            lhsT=vcT_sb[0:3 * W, c2 * P:(c2 + 1) * P],
                            rhs=w3hi[:], start=True, stop=True)
                        nc.tensor.matmul(
                            out=o_ps[:, ob + 3 * W:ob + 6 * W],
                            lhsT=vcT_sb[0:3 * W, (2 + c2) * P:(3 + c2) * P],
                            rhs=w3hi[:], start=True, stop=True)
                        nc.tensor.matmul(
                            out=o_ps[:, ob + 6 * W:ob + 8 * W],
                            lhsT=vcT_sb[0:2 * W, (4 + c2) * P:(5 + c2) * P],
                            rhs=w2hi[:], start=True, stop=True)

                    # PSUM->SBUF; normalization folded here (early) or
                    # pre-applied on Pool (late)
                    fb = fbase + 2 * cp2 * TPC
                    opsv = (o_ps[:].rearrange("p (c h) -> p c h", c=2)
                            [:, :, 0:FO]
                            .rearrange("p c (n e) -> p c n e", e=W))
                    o_out = out_sb[:, 2 * cp2 * FO:(2 * cp2 + 2) * FO].rearrange(
                        "p (c n e) -> p c n e", c=2, e=W)
                    nc.vector.tensor_tensor(
                        out=o_out, in0=opsv,
                        in1=fac[:, fb: fb + 2 * TPC]
                            .rearrange("p (c n) -> p c n", c=2)
                            .unsqueeze(3).to_broadcast([P, 2, TPC, W]),
                        op=OP.mult)
                nc.sync.dma_start(
                    out=out_v[:, g * CPG * FO:(g + 1) * CPG * FO],
                    in_=out_sb[:])

            NW = 2 * CPG * TPC
            GPP = ngroup // 2          # groups per super-phase (4)

            def back_pair(gp, zgate, late=False):
                g0, g1 = 2 * gp, 2 * gp + 1
                expv0, sq0 = back_exp(g0, zgate)
                expv1, sq1 = back_exp(g1, zgate)
                Spair = sp.tile([P, NW], f32, tag="S")
                nc.vector.tensor_reduce(
                    out=Spair[:, 0:CPG * TPC],
                    in_=sq0[:].rearrange("p (n e) -> p n e", e=M),
                    axis=mybir.AxisListType.X, op=OP.add)
                nc.vector.tensor_reduce(
                    out=Spair[:, CPG * TPC:NW],
                    in_=sq1[:].rearrange("p (n e) -> p n e", e=M),
                    axis=mybir.AxisListType.X, op=OP.add)

                # fac = rsqrt(S): bit-hack seed + 2 Newton rounds (DVE)
                fac = sp.tile([P, NW], f32, tag="fac")
                nc.vector.tensor_scalar(
                    out=fac[:].bitcast(u32), in0=Spair[:].bitcast(u32),
                    scalar1=1, scalar2=None, op0=OP.logical_shift_right)
                nc.vector.tensor_scalar(
                    out=fac[:].bitcast(u32), in0=fac[:].bitcast(u32),
                    scalar1=-1.0, scalar2=MAGIC, op0=OP.mult, op1=OP.add)
                for _ in range(2):
                    t2 = sp.tile([P, NW], f32, tag="nt")
                    nc.gpsimd.tensor_tensor(out=t2[:], in0=fac[:], in1=fac[:],
                                            op=OP.mult)
                    nc.gpsimd.tensor_tensor(out=t2[:], in0=t2[:], in1=Spair[:],
                                            op=OP.mult)
                    nc.gpsimd.tensor_scalar(out=t2[:], in0=t2[:], scalar1=-0.5,
                                            scalar2=1.5, op0=OP.mult,
                                            op1=OP.add)
                    nc.gpsimd.tensor_tensor(out=fac[:], in0=fac[:], in1=t2[:],
                                            op=OP.mult)

                back_tail(g0, expv0, fac, 0, late=late)
                back_tail(g1, expv1, fac, CPG * TPC, late=late)
                return sq1

            # Super-phases: back(A) overlaps front(B).  Zero-valued gate tiles
            # sequence the ACT queue (Sin-set ops, then Exp-set ops per phase)
            # so the activation table loads only 2*len(PHASES) times.
            PHASES = [4, 4]          # groups per phase (pairs => even sizes)
            glo = 0
            sbias, cbias = 0.0, halfpi[:]
            for ph, gpp in enumerate(PHASES):
                for g in range(glo, glo + gpp):
                    dma_eng = nc.sync if ph == 0 else nc.scalar
                    front(g, sbias, cbias, dma_eng)
                # gate for this phase's Exp ops: 0-tile reading last cos
                zgate = kp.tile([P, 1], f32, tag=f"zg{ph}", name=f"zg{ph}")
                nc.vector.tensor_scalar(
                    out=zgate[:], in0=cosv[glo + gpp - 1][:, 0:1],
                    scalar1=0.0, scalar2=None, op0=OP.mult)
                last_sq = None
                late = ph == len(PHASES) - 1
                for gp in range(glo // 2, (glo + gpp) // 2):
                    last_sq = back_pair(gp, zgate, late=late)
                glo += gpp
                if ph < len(PHASES) - 1:
                    # gates for next phase's Sin ops: after this phase's Square
                    zsin = kp.tile([P, 1], f32, tag=f"zs{ph}", name=f"zs{ph}")
                    nc.vector.tensor_scalar(
                        out=zsin[:], in0=last_sq[:, 0:1], scalar1=0.0,
                        scalar2=None, op0=OP.mult)
                    ghalfpi = kp.tile([P, 1], f32, tag=f"gh{ph}",
                                      name=f"gh{ph}")
                    nc.vector.tensor_tensor(
                        out=ghalfpi[:], in0=halfpi[:], in1=zsin[:], op=OP.add)
                    sbias, cbias = zsin[:], ghalfpi[:]

    nc.compile()
    return nc


def kernel(x: np.ndarray, shuffle_vector: np.ndarray) -> np.ndarray:
    global _cached
    x = np.asarray(x)
    assert x.shape == (B, K), x.shape
    x_bf = np.ascontiguousarray(x.astype(ml_dtypes.bfloat16))

    tabs = _tables(shuffle_vector)
    if _cached is None:
        _cached = _build_module()
    nc = _cached

    # xt[(t,k), ci*P + p] = x_core[p*CPB + ci*TPC + t, k]
    xs = x_bf.reshape(NCORES, P, NCHUNK, TPC, K)
    in_maps = [
        {"xt": np.ascontiguousarray(
            xs[i].transpose(2, 3, 1, 0).reshape(TPC * K, NCHUNK * P)), **tabs}
        for i in range(NCORES)
    ]
    res = bass_utils.run_bass_kernel_spmd(nc, in_maps, core_ids=list(range(NCORES)))
    out = np.concatenate([res.results[i]["out"] for i in range(NCORES)], axis=0)
    outf = np.ascontiguousarray(out.astype(np.float32))
    return outf.view(np.complex64).reshape(B, M).astype(np.complex128)
